# revision 22
# baseline (speedup 1.0000x reference)
"""Trainium2 Bass kernel for nn_MultiHeadAttention (B=4, S=2048, D=512, H=8).

Sharding: 8 cores = 4 batches x 2 head-groups (4 heads each).
Each core computes, for its (b, hg):
    Q/K/V projections (its 4 heads) -> masked softmax attention -> partial
    output projection  partial_hg = x_hg @ Wo[:, hg_cols].T   (row-sharded).
Host side: inputs are pre-transposed/sliced per core; outputs are summed
across the 2 head-groups per batch and bo is added.

Device layouts (per core), S=2048, D=512, FPC=256 (features per core):
  qT/kT/vT  [D, S]   f32  (host-transposed activations)
  keepT     [S, S]   bf16 (1.0 where attention allowed, 0.0 where masked; [k, q])
  wqT/wkT/wvT [D, FPC] f32 ; woT [FPC, D] f32 (host-transposed weight slices)
  bq/bk [128, 2] f32 ; bv [64, 4] f32  (host-shaped per-partition biases)
  outT      [D, S]   f32  (partial output, transposed)
"""

import sys

if "/opt/trn_rl_repo" not in sys.path:
    sys.path.insert(0, "/opt/trn_rl_repo")

import numpy as np
import ml_dtypes

import concourse.bass as bass
import concourse.mybir as mybir
import concourse.tile as tile
from concourse import bacc
from concourse import bass_utils

F32 = mybir.dt.float32
F32R = mybir.dt.float32r
BF16 = mybir.dt.bfloat16
EXP = mybir.ActivationFunctionType.Exp
LOG = mybir.ActivationFunctionType.Ln
MULT = mybir.AluOpType.mult

B = 4
S = 2048
D = 512
H = 8
DK = 64
HPC = 4          # heads per core
FPC = HPC * DK   # 256 projected features per core
NEG_BIG = -81920.0  # unused on-device (mask is multiplicative) but kept for reference


def build_kernel(s=S, d=D, debug_taps=False):
    """Build and compile the per-core Bass program. Returns compiled nc."""
    n_kt = s // 128          # 128-row k tiles
    n_qc = s // 512          # 512-col q chunks
    n_dc = d // 128          # 128-row d chunks
    n_ft = FPC // 128        # 128-row feature tiles (2)

    nc = bacc.Bacc(
        "TRN2",
        target_bir_lowering=False,
        debug=False,
        enable_asserts=False,
        num_devices=8,
    )

    qT = nc.dram_tensor("qT", [d, s], F32R, kind="ExternalInput").ap()
    kT = nc.dram_tensor("kT", [d, s], F32R, kind="ExternalInput").ap()
    vT = nc.dram_tensor("vT", [d, s], F32R, kind="ExternalInput").ap()
    keepT = nc.dram_tensor("keepT", [s, s], BF16, kind="ExternalInput").ap()
    wqT = nc.dram_tensor("wqT", [d, FPC], F32R, kind="ExternalInput").ap()
    wkT = nc.dram_tensor("wkT", [d, FPC], F32R, kind="ExternalInput").ap()
    wvT = nc.dram_tensor("wvT", [d, FPC], F32R, kind="ExternalInput").ap()
    woT = nc.dram_tensor("woT", [FPC, d], F32R, kind="ExternalInput").ap()
    bq = nc.dram_tensor("bq", [128, n_ft], F32, kind="ExternalInput").ap()
    bk = nc.dram_tensor("bk", [128, n_ft], F32, kind="ExternalInput").ap()
    bv = nc.dram_tensor("bv", [64, HPC], F32, kind="ExternalInput").ap()
    outT = nc.dram_tensor("outT", [d, s], F32, kind="ExternalOutput").ap()
    if debug_taps:
        dbg_QT = nc.dram_tensor("dbg_QT", [128, FPC // 128, s], F32R, kind="ExternalOutput").ap()
        dbg_KT = nc.dram_tensor("dbg_KT", [128, FPC // 128, s], F32R, kind="ExternalOutput").ap()
        dbg_V = nc.dram_tensor("dbg_V", [128, s // 128, HPC * 65], BF16, kind="ExternalOutput").ap()
        dbg_E = nc.dram_tensor("dbg_E", [128, s // 128, 512], BF16, kind="ExternalOutput").ap()
        dbg_x = nc.dram_tensor("dbg_x", [64, HPC, 512], F32R, kind="ExternalOutput").ap()
        dbg_O = nc.dram_tensor("dbg_O", [65, 512], F32, kind="ExternalOutput").ap()
        dbg_rzb = nc.dram_tensor("dbg_rzb", [64, 512], F32, kind="ExternalOutput").ap()
        dbg_rz = nc.dram_tensor("dbg_rz", [512], F32, kind="ExternalOutput").ap()

    with tile.TileContext(nc) as tc:
        with (
            tc.tile_pool(name="weights", bufs=1) as wpool,
            tc.tile_pool(name="resident", bufs=1) as rpool,
        ):
            # ---- resident tensors ----
            # Q_T: [128, n_ft, s]; head h lives at (partitions (h%2)*64.., tile h//2)
            QT_t = rpool.tile([128, n_ft, s], F32R, tag="QT")
            # K_T: same layout as Q_T; head h at (partitions (h%2)*64.., tile h//2)
            KT_t = rpool.tile([128, n_ft, s], F32R, tag="KT")
            # Vaug: [128 (k within tile), n_kt, HPC*65]; per head 64 V cols + ones col.
            Vaug_t = rpool.tile([128, n_kt, HPC * 65], BF16, tag="Vaug")

            wq_t = wpool.tile([128, n_dc, FPC], F32R, tag="wq")
            wk_t = wpool.tile([128, n_dc, FPC], F32R, tag="wk")
            wv_t = wpool.tile([128, n_dc, FPC], F32R, tag="wv")
            wo_t = wpool.tile([64, HPC, d], F32R, tag="wo")
            bq_t = wpool.tile([128, n_ft], F32, tag="bq")
            bk_t = wpool.tile([128, n_ft], F32, tag="bk")
            bv_t = wpool.tile([64, HPC], F32, tag="bv")

            nc.sync.dma_start(wq_t[:], wqT.rearrange("(c p) f -> p c f", p=128))
            nc.sync.dma_start(wk_t[:], wkT.rearrange("(c p) f -> p c f", p=128))
            nc.sync.dma_start(wv_t[:], wvT.rearrange("(c p) f -> p c f", p=128))
            nc.sync.dma_start(wo_t[:], woT.rearrange("(c p) f -> p c f", p=64))
            nc.sync.dma_start(bq_t[:], bq[:])
            nc.sync.dma_start(bk_t[:], bk[:])
            nc.sync.dma_start(bv_t[:], bv[:])

            # ones columns of Vaug
            ones_ap = Vaug_t.rearrange("p t (h c) -> p t h c", c=65)[:, :, :, 64:65]
            nc.vector.memset(ones_ap, 1.0)

            # ================= phase 1: projections =================
            with (
                tc.tile_pool(name="staging", bufs=1) as spool,
                tc.tile_pool(name="p1psum", bufs=4, space="PSUM") as p1psum,
                tc.tile_pool(name="p1psum_v", bufs=2, space="PSUM") as p1psum_v,
            ):
                q_in = spool.tile([128, n_dc, s], F32R, tag="q_in")
                k_in = spool.tile([128, n_dc, s], F32R, tag="k_in")
                v_in = spool.tile([128, n_dc, s], F32R, tag="v_in")
                nc.sync.dma_start(q_in[:], qT.rearrange("(c p) s -> p c s", p=128))
                nc.sync.dma_start(k_in[:], kT.rearrange("(c p) s -> p c s", p=128))
                nc.sync.dma_start(v_in[:], vT.rearrange("(c p) s -> p c s", p=128))

                # Q_T / K_T projections: out[f_tile, s_chunk] = sum_dc wT.T @ xT
                for name, x_in, w_t, b_t in (
                    ("q", q_in, wq_t, bq_t),
                    ("k", k_in, wk_t, bk_t),
                ):
                    for ft in range(n_ft):
                        for sc in range(s // 512):
                            ps = p1psum.tile([128, 512], F32, tag="p1")
                            for dc in range(n_dc):
                                nc.tensor.matmul(
                                    ps[:],
                                    w_t[:, dc, ft * 128 : (ft + 1) * 128],
                                    x_in[:, dc, sc * 512 : (sc + 1) * 512],
                                    start=(dc == 0),
                                    stop=(dc == n_dc - 1),
                                )
                            dst_t = QT_t if name == "q" else KT_t
                            nc.vector.tensor_scalar_add(
                                dst_t[:, ft, sc * 512 : (sc + 1) * 512],
                                ps[:],
                                b_t[:, ft : ft + 1],
                            )

                # V projection -> natural layout [s, FPC], written per s-tile
                for st in range(n_kt):
                    psv = p1psum_v.tile([128, FPC], F32, tag="pv")
                    for dc in range(n_dc):
                        nc.tensor.matmul(
                            psv[:],
                            v_in[:, dc, st * 128 : (st + 1) * 128],
                            wv_t[:, dc, :],
                            start=(dc == 0),
                            stop=(dc == n_dc - 1),
                        )
                    dst = Vaug_t.rearrange("p t (h c) -> p t h c", c=65)[
                        :, st, :, 0:64
                    ]
                    src = psv.rearrange("p (h c) -> p h c", c=64)
                    nc.vector.tensor_copy(dst, src)

            if debug_taps:
                nc.sync.dma_start(dbg_QT[:], QT_t[:])
                nc.sync.dma_start(dbg_KT[:], KT_t[:])
                nc.sync.dma_start(dbg_V[:], Vaug_t[:])

            # ================= phase 2: attention + out projection =================
            GROUPS = []
            kt0 = 0
            while kt0 < n_kt:
                g = min(3, n_kt - kt0)
                GROUPS.append((kt0, g))
                kt0 += g

            with (
                tc.tile_pool(name="keeppool", bufs=2) as keeppool,
                tc.tile_pool(name="epool", bufs=2) as epool,
                tc.tile_pool(name="xpool", bufs=2) as xpool,
                tc.tile_pool(name="zpool", bufs=2) as zpool,
                tc.tile_pool(name="zdram", bufs=2, space="DRAM") as zdram,
                tc.tile_pool(name="outpool", bufs=4) as outpool,
                tc.tile_pool(name="spsum", bufs=2, space="PSUM") as spsum,
                tc.tile_pool(name="opsum", bufs=2, space="PSUM") as opsum,
            ):
                for qc in range(n_qc):
                    qlo, qhi = qc * 512, (qc + 1) * 512
                    keep_t = keeppool.tile([128, n_kt, 512], BF16, tag="keep")
                    nc.sync.dma_start(
                        keep_t[:],
                        keepT.rearrange("(t p) q -> p t q", p=128)[:, :, qlo:qhi],
                    )
                    xT_t = xpool.tile([64, HPC, 512], F32R, tag="xT")

                    for h in range(HPC):
                        ft = h // 2
                        E_t = epool.tile([128, n_kt, 512], BF16, tag="E")
                        # ---- scores + exp, grouped over k tiles ----
                        plo = (h % 2) * 64
                        phi = plo + 64
                        for kt0, g in GROUPS:
                            sp = spsum.tile([128, 3 * 512], F32, tag="S")
                            for i in range(g):
                                ktile = kt0 + i
                                nc.tensor.matmul(
                                    sp[:, i * 512 : (i + 1) * 512],
                                    KT_t[plo:phi, ft, ktile * 128 : (ktile + 1) * 128],
                                    QT_t[plo:phi, ft, qlo:qhi],
                                    start=True,
                                    stop=True,
                                )
                            nc.scalar.activation(
                                E_t[:, kt0 : kt0 + g, :],
                                sp[:, 0 : g * 512],
                                EXP,
                                scale=0.125,
                            )
                        # ---- mask (multiplicative keep) ----
                        half = n_kt // 2
                        nc.vector.tensor_tensor(
                            E_t[:, 0:half, :],
                            E_t[:, 0:half, :],
                            keep_t[:, 0:half, :],
                            MULT,
                        )
                        nc.vector.tensor_tensor(
                            E_t[:, half:n_kt, :],
                            E_t[:, half:n_kt, :],
                            keep_t[:, half:n_kt, :],
                            MULT,
                        )
                        if debug_taps and h == 0 and qc == 0:
                            nc.sync.dma_start(dbg_E[:], E_t[:])
                        # ---- attn @ [V | 1] ----
                        op = opsum.tile([128, 512], F32, tag="O")
                        for ktile in range(n_kt):
                            nc.tensor.matmul(
                                op[0:65, :],
                                Vaug_t[:, ktile, h * 65 : (h + 1) * 65],
                                E_t[:, ktile, :],
                                start=(ktile == 0),
                                stop=(ktile == n_kt - 1),
                            )
                        # ---- normalize + bias ----
                        if debug_taps and h == 0 and qc == 0:
                            dbg_O_sb = zpool.tile([65, 512], F32, tag="dbgO")
                            nc.vector.tensor_copy(dbg_O_sb[:], op[0:65, :])
                            nc.sync.dma_start(dbg_O[:], dbg_O_sb[:])
                        # 1/Z = exp(-ln Z) on ScalarE (one shared table set)
                        rz = zpool.tile([65, 1024], F32, tag="rz")
                        nc.scalar.activation(
                            rz[64:65, 512:1024], op[64:65, :], LOG
                        )
                        nc.scalar.activation(
                            rz[64:65, 0:512], rz[64:65, 512:1024], EXP, scale=-1.0
                        )
                        if debug_taps and h == 0 and qc == 0:
                            nc.sync.dma_start(dbg_rz[:], rz[64:65, 0:512])
                        zd = zdram.tile([512], F32, tag="zd")
                        nc.sync.dma_start(zd[:], rz[64:65, 0:512])
                        rzb = zpool.tile([64, 512], F32, tag="rzb")
                        nc.gpsimd.dma_start(
                            rzb[:], zd[None, :].to_broadcast([64, 512])
                        )
                        if debug_taps and h == 0 and qc == 0:
                            nc.sync.dma_start(dbg_rzb[:], rzb[:])
                        nc.vector.tensor_tensor(
                            xT_t[:, h, :], op[0:64, :], rzb[:], MULT
                        )
                        nc.vector.tensor_scalar_add(
                            xT_t[:, h, :], xT_t[:, h, :], bv_t[:, h : h + 1]
                        )

                    if debug_taps and qc == 0:
                        nc.sync.dma_start(dbg_x[:], xT_t[:])
                    # ---- output projection (row-sharded; host sums partials) ----
                    for ftile in range(d // 128):
                        po = opsum.tile([128, 512], F32, tag="O")
                        for h in range(HPC):
                            nc.tensor.matmul(
                                po[:],
                                wo_t[:, h, ftile * 128 : (ftile + 1) * 128],
                                xT_t[:, h, :],
                                start=(h == 0),
                                stop=(h == HPC - 1),
                            )
                        ot = outpool.tile([128, 512], F32, tag="out")
                        nc.vector.tensor_copy(ot[:], po[:])
                        nc.sync.dma_start(
                            outT[ftile * 128 : (ftile + 1) * 128, qlo:qhi], ot[:]
                        )

    nc.compile()
    return nc


_CACHED_NC = None


def _get_nc():
    global _CACHED_NC
    if _CACHED_NC is None:
        _CACHED_NC = build_kernel()
    return _CACHED_NC


def make_in_maps(query, key, value, mask, Wq, bq, Wk, bk, Wv, bv, Wo, bo):
    n_ft = FPC // 128
    in_maps = []
    for c in range(8):
        b, hg = c // 2, c % 2
        lo, hi = hg * FPC, (hg + 1) * FPC
        keep = (~mask[b]).T.astype(ml_dtypes.bfloat16)  # [k, q]
        m = {
            "qT": np.ascontiguousarray(query[b].T, dtype=np.float32),
            "kT": np.ascontiguousarray(key[b].T, dtype=np.float32),
            "vT": np.ascontiguousarray(value[b].T, dtype=np.float32),
            "keepT": np.ascontiguousarray(keep),
            "wqT": np.ascontiguousarray(Wq[lo:hi].T, dtype=np.float32),
            "wkT": np.ascontiguousarray(Wk[lo:hi].T, dtype=np.float32),
            "wvT": np.ascontiguousarray(Wv[lo:hi].T, dtype=np.float32),
            "woT": np.ascontiguousarray(Wo[:, lo:hi].T, dtype=np.float32),
            "bq": np.ascontiguousarray(
                bq[lo:hi].reshape(n_ft, 128).T, dtype=np.float32
            ),
            "bk": np.ascontiguousarray(
                bk[lo:hi].reshape(n_ft, 128).T, dtype=np.float32
            ),
            "bv": np.ascontiguousarray(
                bv[lo:hi].reshape(HPC, 64).T, dtype=np.float32
            ),
        }
        in_maps.append(m)
    return in_maps


def kernel(query, key, value, mask, Wq, bq, Wk, bk, Wv, bv, Wo, bo):
    query = np.asarray(query, dtype=np.float32)
    key = np.asarray(key, dtype=np.float32)
    value = np.asarray(value, dtype=np.float32)
    mask = np.asarray(mask).astype(bool)
    Wq, bq = np.asarray(Wq, np.float32), np.asarray(bq, np.float32)
    Wk, bk = np.asarray(Wk, np.float32), np.asarray(bk, np.float32)
    Wv, bv = np.asarray(Wv, np.float32), np.asarray(bv, np.float32)
    Wo, bo = np.asarray(Wo, np.float32), np.asarray(bo, np.float32)

    nc = _get_nc()
    in_maps = make_in_maps(
        query, key, value, mask, Wq, bq, Wk, bk, Wv, bv, Wo, bo
    )
    res = bass_utils.run_bass_kernel_spmd(nc, in_maps, core_ids=list(range(8)))

    out = np.empty((B, S, D), dtype=np.float32)
    for b in range(B):
        partial = res.results[2 * b]["outT"] + res.results[2 * b + 1]["outT"]
        out[b] = partial.T + bo[None, :]
    return out
